# revision 10
# baseline (speedup 1.0000x reference)
"""DenseCapsule dynamic-routing kernel for 8 Trainium2 NeuronCores.

Problem: x[B=32,I=2048,D=16], w_ij[J=64,I=2048,C=32,D=16]
  u_hat = einsum('bid,jicd->bjic', x, w_ij)
  5 routing iterations (softmax over J, s = sum_i c*u_hat, v = squash(s),
  b += sum_c v*u_hat), return v [B,J,C].

Sharding: input capsules I are split 8 ways (I_LOC=256 per core).  The
softmax over J is then core-local; the only collective is an AllReduce of
the per-core partial s [B,J,C] (256 KB) once per iteration.

v2 layout: everything is c-major — u_hat tiles are [128=(g4,b32) parts,
(c32,j64)-ordered free] so the softmax coefficient c[b,j,i] (constant
over c) can be read by the DVE prod2 multiply as a stride-0-middle
broadcast AP instead of being materialized J->J*C on ACT.  1/Z is folded
into the coefficient, so every phase-2 matmul shares the same d1_t
stationary.  exp's accum_out produces Z for free.  The logits reduction
uses a (c, n*j)-ordered product tile so the c-tree is 5 contiguous
2x-mode adds.  u_hat is stored in DRAM as fp16.
"""

import numpy as np

B, I, D, J, C = 32, 2048, 16, 64, 32
NCORES = 8
I_LOC = I // NCORES      # 256
G = 4                    # i's per block (G*D = 64 contraction partitions)
NBLK = I_LOC // G        # 64
JC = J * C               # 2048
ITERS = 5
EPS = 1e-7
GR = 8                   # i-blocks per phase-2 tile group

_CACHE = {}


def _build(repeats=1):
    import concourse.bacc as bacc
    import concourse.mybir as mybir
    from concourse import tile

    f32 = mybir.dt.float32
    fp16 = mybir.dt.float16
    Act = mybir.ActivationFunctionType
    Alu = mybir.AluOpType
    AxX = mybir.AxisListType.X

    nc = bacc.Bacc("TRN2", target_bir_lowering=False, debug=False,
                   num_devices=NCORES)
    xd = nc.dram_tensor("xd", [NBLK, G * D, 128], fp16, kind="ExternalInput").ap()
    wm = nc.dram_tensor("wm", [NBLK, G * D, JC], fp16, kind="ExternalInput").ap()
    d1 = nc.dram_tensor("d1", [128, B], fp16, kind="ExternalInput").ap()
    v_out = nc.dram_tensor("v_out", [B, JC], f32, kind="ExternalOutput").ap()

    with tile.TileContext(nc) as tc:
        with tc.tile_pool(name="const", bufs=1) as constp, \
             tc.tile_pool(name="io", bufs=2) as iop, \
             tc.tile_pool(name="u", bufs=1) as up, \
             tc.tile_pool(name="work", bufs=2) as wp, \
             tc.tile_pool(name="small", bufs=1) as sp, \
             tc.tile_pool(name="spg", bufs=3) as spg, \
             tc.tile_pool(name="psum", bufs=4, space="PSUM") as pp, \
             tc.tile_pool(name="spsum", bufs=1, space="PSUM") as spp, \
             tc.tile_pool(name="ud", bufs=1, space="DRAM") as udp, \
             tc.tile_pool(name="ar", bufs=2, space="DRAM") as arp:

            d1_t = constp.tile([128, B], fp16)
            nc.sync.dma_start(d1_t[:], d1[:])
            b_tiles = []                                 # routing logits
            for gi in range(NBLK // GR):
                bt = constp.tile([128, GR * J], f32, tag=f"b{gi}")
                b_tiles.append(bt)
            u_store = udp.tile([NBLK, 128, JC], fp16)

            for _rep in range(repeats):
                for bt in b_tiles:
                    nc.gpsimd.memset(bt[:], 0.0)

                # ---- Phase 1: u_hat production + iteration-1 s accumulation
                s_ps = spp.tile([B, JC], f32, tag="s")
                for blk in range(NBLK):
                    xd_t = iop.tile([G * D, 128], fp16, tag="xd_t")
                    nc.sync.dma_start(xd_t[:], xd[blk])
                    wm_t = iop.tile([G * D, JC], fp16, tag="wm_t")
                    nc.sync.dma_start(wm_t[:], wm[blk])
                    u16 = iop.tile([128, JC], fp16, tag="u16")
                    for ch in range(4):
                        sl = slice(ch * 512, (ch + 1) * 512)
                        ps = pp.tile([128, 512], f32, tag="ps")
                        nc.tensor.matmul(ps[:], xd_t[:], wm_t[:, sl],
                                         start=True, stop=True)
                        if ch % 2 == 0:
                            nc.vector.tensor_copy(u16[:, sl], ps[:])
                        else:
                            nc.scalar.copy(u16[:, sl], ps[:])
                    nc.sync.dma_start(u_store[blk], u16[:])
                    for ch in range(4):
                        sl = slice(ch * 512, (ch + 1) * 512)
                        nc.tensor.matmul(s_ps[:, sl], d1_t[:], u16[:, sl],
                                         start=(blk == 0), stop=(blk == NBLK - 1))

                # ---- Phase 2: routing iterations
                for it in range(1, ITERS + 1):
                    last = it == ITERS
                    # v^{it} from the s accumulated for iteration `it`;
                    # the collective payload is fp16 (halves AllReduce and
                    # lets the squash chain run in DVE 2x mode)
                    s_sb = sp.tile([B, JC], fp16, tag="s_sb")
                    nc.scalar.activation(s_sb[:], s_ps[:], Act.Copy, bias=0.0,
                                         scale=(1.0 / J) if it == 1 else 1.0)
                    ar_in = arp.tile([B, JC], fp16, tag="ar_in")
                    ar_out = arp.tile([B, JC], fp16, tag="ar_out")
                    nc.sync.dma_start(ar_in[:], s_sb[:])
                    nc.gpsimd.collective_compute(
                        "AllReduce", Alu.add,
                        replica_groups=[list(range(NCORES))],
                        ins=[ar_in.opt()], outs=[ar_out.opt()],
                    )
                    s_full = sp.tile([B, JC], fp16, tag="s_full")
                    nc.sync.dma_start(s_full[:], ar_out[:])

                    if not last:
                        # prefetch the first two u_t groups while the
                        # squash chain runs
                        pre_ut = []
                        for g0 in (0, GR):
                            u_t = up.tile([128, GR * JC], fp16, tag="u_t",
                                          bufs=2)
                            nc.sync.dma_start(
                                u_t[:].rearrange("p (n f) -> p n f", n=GR),
                                u_store[g0:g0 + GR].rearrange(
                                    "n p f -> p n f"))
                            pre_ut.append(u_t)

                    # squash: v0 = s+eps; scale = sqrt(n)/(1+n), n = sum_c v0^2
                    sqdt = f32 if last else fp16
                    v0 = sp.tile([B, JC], sqdt, tag="v0")
                    nc.vector.tensor_scalar_add(v0[:], s_full[:], EPS)
                    sq = sp.tile([B, JC], sqdt, tag="sq")
                    nc.vector.tensor_mul(sq[:], v0[:], v0[:])
                    q3 = sq[:].rearrange("p (c j) -> p c j", c=C)
                    for tw in (16, 8, 4, 2):
                        nc.vector.tensor_add(q3[:, 0:tw, :], q3[:, 0:tw, :],
                                             q3[:, tw:2 * tw, :])
                    norm = sp.tile([B, J], f32, tag="norm")
                    nc.vector.tensor_add(
                        norm[:].rearrange("p (o j) -> p o j", o=1),
                        q3[:, 0:1, :], q3[:, 1:2, :])
                    rt = sp.tile([B, J], f32, tag="rt")
                    nc.scalar.activation(rt[:], norm[:], Act.Sqrt)
                    np1 = sp.tile([B, J], f32, tag="np1")
                    nc.vector.tensor_scalar_add(np1[:], norm[:], 1.0)
                    inv1 = sp.tile([B, J], f32, tag="inv1")
                    nc.vector.reciprocal(inv1[:], np1[:])
                    invd = sp.tile([B, J], f32, tag="invd")
                    nc.vector.tensor_mul(invd[:], rt[:], inv1[:])
                    if last:
                        v_sb = v0    # in-place: v0's last use
                        nc.vector.tensor_mul(
                            v_sb[:].rearrange("p (c j) -> p c j", c=C),
                            v0[:].rearrange("p (c j) -> p c j", c=C),
                            invd[:].rearrange("p (o j) -> p o j", o=1)
                                   .broadcast_to((B, C, J)))
                        nc.sync.dma_start(v_out[:], v_sb[:])
                        break
                    invd16 = sp.tile([B, J], fp16, tag="invd16")
                    nc.scalar.copy(invd16[:], invd[:])
                    v_sb = v0    # in-place: v0's last use
                    nc.vector.tensor_mul(
                        v_sb[:].rearrange("p (c j) -> p c j", c=C),
                        v0[:].rearrange("p (c j) -> p c j", c=C),
                        invd16[:].rearrange("p (o j) -> p o j", o=1)
                                 .broadcast_to((B, C, J)))

                    v_rep = constp.tile([128, JC], fp16, tag="v_rep")
                    for g in range(G):
                        nc.sync.dma_start(v_rep[g * B:(g + 1) * B, :], v_sb[:])

                    s_ps = spp.tile([B, JC], f32, tag="s")

                    def stage_b(u_t, c_grp, g0):
                        # s += c * u: coefficient read as a stride-0-middle
                        # broadcast over c; matmul stationary is always d1_t
                        u4 = u_t[:].rearrange("p (n c j) -> p n c j",
                                              n=GR, c=C)
                        for n in range(GR):
                            nc.vector.tensor_mul(
                                u4[:, n], u4[:, n],
                                c_grp[:, n * J:(n + 1) * J]
                                    .rearrange("p (o j) -> p o j", o=1)
                                    .broadcast_to((128, C, J)))
                        uf = u_t[:].rearrange("p (n f) -> p n f", n=GR)
                        for n in range(GR):
                            blk = g0 + n
                            for ch in range(4):
                                sl = slice(ch * 512, (ch + 1) * 512)
                                nc.tensor.matmul(s_ps[:, sl], d1_t[:],
                                                 uf[:, n, sl],
                                                 start=(blk == 0),
                                                 stop=(blk == NBLK - 1))

                    pending = None
                    for g0 in range(0, NBLK, GR):
                        b_g = b_tiles[g0 // GR]
                        if g0 // GR < len(pre_ut):
                            u_t = pre_ut[g0 // GR]
                        else:
                            u_t = up.tile([128, GR * JC], fp16, tag="u_t",
                                          bufs=2)
                            nc.sync.dma_start(
                                u_t[:].rearrange("p (n f) -> p n f", n=GR),
                                u_store[g0:g0 + GR].rearrange("n p f -> p n f"))
                        # logits update t = sum_c u*v: product stored
                        # (c, n*j)-ordered so the c-tree is contiguous adds
                        prod = wp.tile([128, GR * JC], fp16, tag="prod",
                                       bufs=2)
                        p4 = prod[:].rearrange("p (c n j) -> p c n j",
                                               c=C, n=GR)
                        u4 = u_t[:].rearrange("p (n c j) -> p n c j",
                                              n=GR, c=C)
                        vr = v_rep[:].rearrange("p (c j) -> p c j", c=C)
                        for n in range(GR):
                            nc.vector.tensor_mul(p4[:, :, n, :], u4[:, n], vr)
                        p3 = prod[:].rearrange("p (c s) -> p c s", c=C)
                        for tw in (16, 8, 4, 2):
                            nc.vector.tensor_add(p3[:, 0:tw, :], p3[:, 0:tw, :],
                                                 p3[:, tw:2 * tw, :])
                        t32 = spg.tile([128, GR * J], f32, tag="t32")
                        nc.vector.tensor_add(
                            t32[:].rearrange("p (o s) -> p o s", o=1),
                            p3[:, 0:1, :], p3[:, 1:2, :])
                        nc.gpsimd.tensor_add(b_g[:], b_g[:], t32[:])
                        # core-local softmax over j; coefficient = e/Z in
                        # fp16, Z via exp's accum_out, 1/Z folded on Pool
                        nmx = spg.tile([128, GR], f32, tag="nmx")
                        nc.vector.reduce_max(
                            nmx[:], b_g[:].rearrange("p (n j) -> p n j", n=GR),
                            axis=AxX, negate=True)
                        # c = exp(b - max - lnZ): first exp pass only for Z
                        # (via accum_out), second pass emits c directly, all
                        # on the otherwise-idle ACT engine
                        c_grp = spg.tile([128, GR * J], fp16, tag="c_grp",
                                         bufs=2)
                        zg = spg.tile([128, GR], f32, tag="zg")
                        for n in range(GR):
                            nc.scalar.activation(
                                c_grp[:, n * J:(n + 1) * J],
                                b_g[:, n * J:(n + 1) * J],
                                Act.Exp, bias=nmx[:, n:n + 1],
                                accum_out=zg[:, n:n + 1])
                        lnz = spg.tile([128, GR], f32, tag="lnz")
                        nc.scalar.activation(lnz[:], zg[:], Act.Ln)
                        bias2 = spg.tile([128, GR], f32, tag="bias2")
                        nc.vector.tensor_sub(bias2[:], nmx[:], lnz[:])
                        for n in range(GR):
                            nc.scalar.activation(
                                c_grp[:, n * J:(n + 1) * J],
                                b_g[:, n * J:(n + 1) * J],
                                Act.Exp, bias=bias2[:, n:n + 1])
                        # software pipeline: emit the previous group's
                        # prod2+matmuls after this group's A-stage so DVE/PE
                        # have ready work while ACT/Pool finish the softmax
                        if pending is not None:
                            stage_b(*pending)
                        pending = (u_t, c_grp, g0)
                    stage_b(*pending)

    nc.compile()
    return nc


def _prep_inputs(x, w_ij):
    """Host-side shard + layout. Returns per-core in_maps."""
    x_t = np.ascontiguousarray(x.transpose(1, 2, 0)).astype(np.float16)   # [I,D,B]
    # c-major: [I, D, C, J]
    w_t = np.ascontiguousarray(w_ij.transpose(1, 3, 2, 0)).astype(np.float16)
    d1 = np.tile(np.eye(B, dtype=np.float16), (G, 1))                     # [128,B]
    in_maps = []
    for k in range(NCORES):
        xs = x_t[k * I_LOC:(k + 1) * I_LOC].reshape(NBLK, G, D, B)
        xd = np.zeros((NBLK, G * D, 128), np.float16)
        for g in range(G):
            xd[:, g * D:(g + 1) * D, g * B:(g + 1) * B] = xs[:, g]
        ws = w_t[k * I_LOC:(k + 1) * I_LOC].reshape(NBLK, G * D, JC)
        in_maps.append({"xd": xd, "wm": np.ascontiguousarray(ws), "d1": d1})
    return in_maps


def kernel(x, w_ij, _trace=False):
    from concourse import bass_utils

    if "nc" not in _CACHE:
        _CACHE["nc"] = _build()
    nc = _CACHE["nc"]
    in_maps = _prep_inputs(np.asarray(x), np.asarray(w_ij))
    res = bass_utils.run_bass_kernel_spmd(
        nc, in_maps, core_ids=list(range(NCORES)), trace=_trace)
    _CACHE["last_result"] = res
    v = res.results[0]["v_out"].reshape(B, C, J).transpose(0, 2, 1)
    return np.ascontiguousarray(v.astype(np.float32))


# revision 11
# speedup vs baseline: 1.1062x; 1.1062x over previous
"""DenseCapsule dynamic-routing kernel for 8 Trainium2 NeuronCores.

Problem: x[B=32,I=2048,D=16], w_ij[J=64,I=2048,C=32,D=16]
  u_hat = einsum('bid,jicd->bjic', x, w_ij)
  5 routing iterations (softmax over J, s = sum_i c*u_hat, v = squash(s),
  b += sum_c v*u_hat), return v [B,J,C].

Sharding: input capsules I are split 8 ways (I_LOC=256 per core).  The
softmax over J is then core-local; the only collective is an AllReduce of
the per-core partial s [B,J,C] (256 KB) once per iteration.

v2 layout: everything is c-major — u_hat tiles are [128=(g4,b32) parts,
(c32,j64)-ordered free] so the softmax coefficient c[b,j,i] (constant
over c) can be read by the DVE prod2 multiply as a stride-0-middle
broadcast AP instead of being materialized J->J*C on ACT.  1/Z is folded
into the coefficient, so every phase-2 matmul shares the same d1_t
stationary.  exp's accum_out produces Z for free.  The logits reduction
uses a (c, n*j)-ordered product tile so the c-tree is 5 contiguous
2x-mode adds.  u_hat is stored in DRAM as fp16.
"""

import numpy as np

B, I, D, J, C = 32, 2048, 16, 64, 32
NCORES = 8
I_LOC = I // NCORES      # 256
G = 4                    # i's per block (G*D = 64 contraction partitions)
NBLK = I_LOC // G        # 64
JC = J * C               # 2048
ITERS = 5
EPS = 1e-7
GR = 4                   # i-blocks per phase-2 tile group

_CACHE = {}


def _build(repeats=1):
    import concourse.bacc as bacc
    import concourse.mybir as mybir
    from concourse import tile

    f32 = mybir.dt.float32
    fp16 = mybir.dt.float16
    Act = mybir.ActivationFunctionType
    Alu = mybir.AluOpType
    AxX = mybir.AxisListType.X

    nc = bacc.Bacc("TRN2", target_bir_lowering=False, debug=False,
                   num_devices=NCORES)
    xd = nc.dram_tensor("xd", [NBLK, G * D, 128], fp16, kind="ExternalInput").ap()
    wm = nc.dram_tensor("wm", [NBLK, G * D, JC], fp16, kind="ExternalInput").ap()
    d1 = nc.dram_tensor("d1", [128, B], fp16, kind="ExternalInput").ap()
    v_out = nc.dram_tensor("v_out", [B, JC], f32, kind="ExternalOutput").ap()

    with tile.TileContext(nc) as tc:
        with tc.tile_pool(name="const", bufs=1) as constp, \
             tc.tile_pool(name="io", bufs=2) as iop, \
             tc.tile_pool(name="u", bufs=1) as up, \
             tc.tile_pool(name="work", bufs=2) as wp, \
             tc.tile_pool(name="small", bufs=1) as sp, \
             tc.tile_pool(name="spg", bufs=3) as spg, \
             tc.tile_pool(name="psum", bufs=4, space="PSUM") as pp, \
             tc.tile_pool(name="spsum", bufs=1, space="PSUM") as spp, \
             tc.tile_pool(name="ud", bufs=1, space="DRAM") as udp, \
             tc.tile_pool(name="ar", bufs=2, space="DRAM") as arp:

            d1_t = constp.tile([128, B], fp16)
            nc.sync.dma_start(d1_t[:], d1[:])
            b_tiles = []                                 # routing logits
            for gi in range(NBLK // GR):
                bt = constp.tile([128, GR * J], f32, tag=f"b{gi}")
                b_tiles.append(bt)
            u_store = udp.tile([NBLK, 128, JC], fp16)

            for _rep in range(repeats):
                for bt in b_tiles:
                    nc.gpsimd.memset(bt[:], 0.0)

                # ---- Phase 1: u_hat production + iteration-1 s accumulation
                s_ps = spp.tile([B, JC], f32, tag="s")
                for blk in range(NBLK):
                    xd_t = iop.tile([G * D, 128], fp16, tag="xd_t")
                    nc.sync.dma_start(xd_t[:], xd[blk])
                    wm_t = iop.tile([G * D, JC], fp16, tag="wm_t")
                    nc.sync.dma_start(wm_t[:], wm[blk])
                    u16 = iop.tile([128, JC], fp16, tag="u16")
                    for ch in range(4):
                        sl = slice(ch * 512, (ch + 1) * 512)
                        ps = pp.tile([128, 512], f32, tag="ps")
                        nc.tensor.matmul(ps[:], xd_t[:], wm_t[:, sl],
                                         start=True, stop=True)
                        if ch % 2 == 0:
                            nc.vector.tensor_copy(u16[:, sl], ps[:])
                        else:
                            nc.scalar.copy(u16[:, sl], ps[:])
                    nc.sync.dma_start(u_store[blk], u16[:])
                    for ch in range(4):
                        sl = slice(ch * 512, (ch + 1) * 512)
                        nc.tensor.matmul(s_ps[:, sl], d1_t[:], u16[:, sl],
                                         start=(blk == 0), stop=(blk == NBLK - 1))

                # ---- Phase 2: routing iterations
                for it in range(1, ITERS + 1):
                    last = it == ITERS
                    # v^{it} from the s accumulated for iteration `it`;
                    # the collective payload is fp16 (halves AllReduce and
                    # lets the squash chain run in DVE 2x mode)
                    s_sb = sp.tile([B, JC], fp16, tag="s_sb")
                    nc.scalar.activation(s_sb[:], s_ps[:], Act.Copy, bias=0.0,
                                         scale=(1.0 / J) if it == 1 else 1.0)
                    ar_in = arp.tile([B, JC], fp16, tag="ar_in")
                    ar_out = arp.tile([B, JC], fp16, tag="ar_out")
                    nc.sync.dma_start(ar_in[:], s_sb[:])
                    nc.gpsimd.collective_compute(
                        "AllReduce", Alu.add,
                        replica_groups=[list(range(NCORES))],
                        ins=[ar_in.opt()], outs=[ar_out.opt()],
                    )
                    s_full = sp.tile([B, JC], fp16, tag="s_full")
                    nc.sync.dma_start(s_full[:], ar_out[:])

                    if not last:
                        # prefetch the first two u_t groups while the
                        # squash chain runs
                        pre_ut = []
                        for g0 in (0, GR):
                            u_t = up.tile([128, GR * JC], fp16, tag="u_t",
                                          bufs=3)
                            nc.sync.dma_start(
                                u_t[:].rearrange("p (n f) -> p n f", n=GR),
                                u_store[g0:g0 + GR].rearrange(
                                    "n p f -> p n f"))
                            pre_ut.append(u_t)

                    # squash: v0 = s+eps; scale = sqrt(n)/(1+n), n = sum_c v0^2
                    sqdt = f32 if last else fp16
                    v0 = sp.tile([B, JC], sqdt, tag="v0")
                    nc.vector.tensor_scalar_add(v0[:], s_full[:], EPS)
                    sq = sp.tile([B, JC], sqdt, tag="sq")
                    nc.vector.tensor_mul(sq[:], v0[:], v0[:])
                    q3 = sq[:].rearrange("p (c j) -> p c j", c=C)
                    for tw in (16, 8, 4, 2):
                        nc.vector.tensor_add(q3[:, 0:tw, :], q3[:, 0:tw, :],
                                             q3[:, tw:2 * tw, :])
                    norm = sp.tile([B, J], f32, tag="norm")
                    nc.vector.tensor_add(
                        norm[:].rearrange("p (o j) -> p o j", o=1),
                        q3[:, 0:1, :], q3[:, 1:2, :])
                    rt = sp.tile([B, J], f32, tag="rt")
                    nc.scalar.activation(rt[:], norm[:], Act.Sqrt)
                    np1 = sp.tile([B, J], f32, tag="np1")
                    nc.vector.tensor_scalar_add(np1[:], norm[:], 1.0)
                    inv1 = sp.tile([B, J], f32, tag="inv1")
                    nc.vector.reciprocal(inv1[:], np1[:])
                    invd = sp.tile([B, J], f32, tag="invd")
                    nc.vector.tensor_mul(invd[:], rt[:], inv1[:])
                    if last:
                        v_sb = v0    # in-place: v0's last use
                        nc.vector.tensor_mul(
                            v_sb[:].rearrange("p (c j) -> p c j", c=C),
                            v0[:].rearrange("p (c j) -> p c j", c=C),
                            invd[:].rearrange("p (o j) -> p o j", o=1)
                                   .broadcast_to((B, C, J)))
                        nc.sync.dma_start(v_out[:], v_sb[:])
                        break
                    invd16 = sp.tile([B, J], fp16, tag="invd16")
                    nc.scalar.copy(invd16[:], invd[:])
                    v_sb = v0    # in-place: v0's last use
                    nc.vector.tensor_mul(
                        v_sb[:].rearrange("p (c j) -> p c j", c=C),
                        v0[:].rearrange("p (c j) -> p c j", c=C),
                        invd16[:].rearrange("p (o j) -> p o j", o=1)
                                 .broadcast_to((B, C, J)))

                    v_rep = constp.tile([128, JC], fp16, tag="v_rep")
                    for g in range(G):
                        nc.sync.dma_start(v_rep[g * B:(g + 1) * B, :], v_sb[:])

                    s_ps = spp.tile([B, JC], f32, tag="s")

                    def stage_b(u_t, c_grp, g0):
                        # s += c * u: coefficient read as a stride-0-middle
                        # broadcast over c; matmul stationary is always d1_t
                        u4 = u_t[:].rearrange("p (n c j) -> p n c j",
                                              n=GR, c=C)
                        for n in range(GR):
                            nc.vector.tensor_mul(
                                u4[:, n], u4[:, n],
                                c_grp[:, n * J:(n + 1) * J]
                                    .rearrange("p (o j) -> p o j", o=1)
                                    .broadcast_to((128, C, J)))
                        uf = u_t[:].rearrange("p (n f) -> p n f", n=GR)
                        for n in range(GR):
                            blk = g0 + n
                            for ch in range(4):
                                sl = slice(ch * 512, (ch + 1) * 512)
                                nc.tensor.matmul(s_ps[:, sl], d1_t[:],
                                                 uf[:, n, sl],
                                                 start=(blk == 0),
                                                 stop=(blk == NBLK - 1))

                    pending = None
                    for g0 in range(0, NBLK, GR):
                        b_g = b_tiles[g0 // GR]
                        if g0 // GR < len(pre_ut):
                            u_t = pre_ut[g0 // GR]
                        else:
                            u_t = up.tile([128, GR * JC], fp16, tag="u_t",
                                          bufs=3)
                            nc.sync.dma_start(
                                u_t[:].rearrange("p (n f) -> p n f", n=GR),
                                u_store[g0:g0 + GR].rearrange("n p f -> p n f"))
                        # logits update t = sum_c u*v: product stored
                        # (c, n*j)-ordered so the c-tree is contiguous adds
                        prod = wp.tile([128, GR * JC], fp16, tag="prod",
                                       bufs=2)
                        p4 = prod[:].rearrange("p (c n j) -> p c n j",
                                               c=C, n=GR)
                        u4 = u_t[:].rearrange("p (n c j) -> p n c j",
                                              n=GR, c=C)
                        vr = v_rep[:].rearrange("p (c j) -> p c j", c=C)
                        for n in range(GR):
                            nc.vector.tensor_mul(p4[:, :, n, :], u4[:, n], vr)
                        p3 = prod[:].rearrange("p (c s) -> p c s", c=C)
                        for tw in (16, 8, 4, 2):
                            nc.vector.tensor_add(p3[:, 0:tw, :], p3[:, 0:tw, :],
                                                 p3[:, tw:2 * tw, :])
                        t32 = spg.tile([128, GR * J], f32, tag="t32")
                        nc.vector.tensor_add(
                            t32[:].rearrange("p (o s) -> p o s", o=1),
                            p3[:, 0:1, :], p3[:, 1:2, :])
                        nc.gpsimd.tensor_add(b_g[:], b_g[:], t32[:])
                        # core-local softmax over j; coefficient = e/Z in
                        # fp16, Z via exp's accum_out, 1/Z folded on Pool
                        nmx = spg.tile([128, GR], f32, tag="nmx")
                        nc.vector.reduce_max(
                            nmx[:], b_g[:].rearrange("p (n j) -> p n j", n=GR),
                            axis=AxX, negate=True)
                        # c = exp(b - max - lnZ): first exp pass only for Z
                        # (via accum_out), second pass emits c directly, all
                        # on the otherwise-idle ACT engine
                        c_grp = spg.tile([128, GR * J], fp16, tag="c_grp",
                                         bufs=2)
                        zg = spg.tile([128, GR], f32, tag="zg")
                        for n in range(GR):
                            nc.scalar.activation(
                                c_grp[:, n * J:(n + 1) * J],
                                b_g[:, n * J:(n + 1) * J],
                                Act.Exp, bias=nmx[:, n:n + 1],
                                accum_out=zg[:, n:n + 1])
                        lnz = spg.tile([128, GR], f32, tag="lnz")
                        nc.scalar.activation(lnz[:], zg[:], Act.Ln)
                        bias2 = spg.tile([128, GR], f32, tag="bias2")
                        nc.vector.tensor_sub(bias2[:], nmx[:], lnz[:])
                        for n in range(GR):
                            nc.scalar.activation(
                                c_grp[:, n * J:(n + 1) * J],
                                b_g[:, n * J:(n + 1) * J],
                                Act.Exp, bias=bias2[:, n:n + 1])
                        # software pipeline: emit the previous group's
                        # prod2+matmuls after this group's A-stage so DVE/PE
                        # have ready work while ACT/Pool finish the softmax
                        if pending is not None:
                            stage_b(*pending)
                        pending = (u_t, c_grp, g0)
                    stage_b(*pending)

    nc.compile()
    return nc


def _prep_inputs(x, w_ij):
    """Host-side shard + layout. Returns per-core in_maps."""
    x_t = np.ascontiguousarray(x.transpose(1, 2, 0)).astype(np.float16)   # [I,D,B]
    # c-major: [I, D, C, J]
    w_t = np.ascontiguousarray(w_ij.transpose(1, 3, 2, 0)).astype(np.float16)
    d1 = np.tile(np.eye(B, dtype=np.float16), (G, 1))                     # [128,B]
    in_maps = []
    for k in range(NCORES):
        xs = x_t[k * I_LOC:(k + 1) * I_LOC].reshape(NBLK, G, D, B)
        xd = np.zeros((NBLK, G * D, 128), np.float16)
        for g in range(G):
            xd[:, g * D:(g + 1) * D, g * B:(g + 1) * B] = xs[:, g]
        ws = w_t[k * I_LOC:(k + 1) * I_LOC].reshape(NBLK, G * D, JC)
        in_maps.append({"xd": xd, "wm": np.ascontiguousarray(ws), "d1": d1})
    return in_maps


def kernel(x, w_ij, _trace=False):
    from concourse import bass_utils

    if "nc" not in _CACHE:
        _CACHE["nc"] = _build()
    nc = _CACHE["nc"]
    in_maps = _prep_inputs(np.asarray(x), np.asarray(w_ij))
    res = bass_utils.run_bass_kernel_spmd(
        nc, in_maps, core_ids=list(range(NCORES)), trace=_trace)
    _CACHE["last_result"] = res
    v = res.results[0]["v_out"].reshape(B, C, J).transpose(0, 2, 1)
    return np.ascontiguousarray(v.astype(np.float32))
